# revision 1
# baseline (speedup 1.0000x reference)
"""DenseAttention (causal quadratic variant, no softmax) — TRN2 Bass kernel.

Problem: out[b] = (tril(Q @ K^T) @ V) per head, where
  Q = X @ Wq (split into 16 heads of 64), K = V = X head slices.
Shapes: X [2, 2048, 1024] fp32, Wq [1024, 1024] fp32 -> out [2, 2048, 1024] fp32.

Sharding (8 cores): core c -> batch b = c//4, head group g = c%4 (4 heads,
output columns [256g, 256g+256)).  The queries projection is column-sharded
by head group; no cross-device communication.

Algorithm per core (linear-attention prefix-sum form, per head h):
  attn_I = Q_I @ S_{<I} + tril(Q_I @ K_I^T) @ V_I      (blocks I of 256 rows)
  S_I = S_{<I} + sum over 128-blocks j in I of K_j^T @ V_j   ([64,64] state)
This reduces the strictly-causal off-diagonal work from O(N^2 hd) to O(N hd^2).
Everything is computed transposed (attnT [hd, N]) so both matmul stages feed
the tensor engine without any on-device transposes; the host un-transposes.

All matmuls run in bf16 with fp32 PSUM accumulation (validated ~2.8e-3 rel
error vs the fp32 reference in numpy emulation).
"""

import numpy as np
import ml_dtypes

import concourse.bacc as bacc
import concourse.mybir as mybir
import concourse.tile as tile
from concourse import bass_utils
from concourse.bass import ds, ts

B, N, D = 2, 2048, 1024
H, HD = 16, 64
NCORES = 8
P = 128           # partition dim
NQ = 256          # q-block (outer) size
T = N // NQ       # 8 outer blocks
KB = N // P       # 16 k-blocks
CW = 256          # per-core output column width (4 heads)

DT = mybir.dt.bfloat16
NPDT = ml_dtypes.bfloat16
F32 = mybir.dt.float32


def _emit(nc, tc, xt_d, wq_d, xv_d, mk_d, out_d):
    with (
        tc.tile_pool(name="const", bufs=1) as cpool,
        tc.tile_pool(name="work", bufs=8) as wpool,
        tc.tile_pool(name="psq", bufs=1, space="PSUM") as psq,
        tc.tile_pool(name="psat", bufs=3, space="PSUM") as psat,
    ):
        # ---------------- input DMAs: few, large, fully-contiguous transfers.
        # The host ships every input already in its SBUF layout (8 KB
        # contiguous per partition row), so each DMA is a plain row-slice
        # copy.  mask/wq go on the ACT HWDGE queue, xv/xt on the SP queue.
        # xt arrives in 512-column chunks (all 8 k-tiles per chunk,
        # [p, (c k w)] layout), chunk-major, so Q-proj chunk c (and the
        # attention blocks it unlocks) only waits for (c+1)/4 of the traffic.
        wqall = cpool.tile([P, 8 * CW], DT, name="wqall", tag="wqall")
        nc.scalar.dma_start(out=wqall, in_=wq_d)

        mk_sb = cpool.tile([P, 2 * NQ], DT, name="mk_sb", tag="mk_sb")
        nc.scalar.dma_start(out=mk_sb, in_=mk_d)

        # xv quarters stream on the SP queue (feeding the S phase) while the
        # ACT queue — idle after wq/mask — carries xt chunk 0 in parallel,
        # so Q-proj c=0 can start right as the S phase drains (the two
        # HWDGE queues are physically parallel on HW).
        xvall = cpool.tile([P, KB * CW], DT, name="xvall", tag="xvall")
        xtall = cpool.tile([P, 8 * N], DT, name="xtall", tag="xtall")
        nc.sync.dma_start(out=xvall[:, ds(0, 1024)], in_=xv_d[:, ds(0, 1024)])
        nc.scalar.dma_start(out=xtall[:, ds(0, 4096)], in_=xt_d[:, ds(0, 4096)])
        for h in range(1, 4):
            nc.sync.dma_start(
                out=xvall[:, ds(1024 * h, 1024)],
                in_=xv_d[:, ds(1024 * h, 1024)],
            )
        # chunk 1 also rides the ACT queue (parallel with the xv stream on
        # SP), chunks 2-3 on SP behind xv
        nc.scalar.dma_start(out=xtall[:, ds(4096, 4096)], in_=xt_d[:, ds(4096, 4096)])
        for c in range(2, 4):
            nc.sync.dma_start(
                out=xtall[:, ds(4096 * c, 4096)],
                in_=xt_d[:, ds(4096 * c, 4096)],
            )

        def xt_ap(k, col, w):
            # xtall layout: [p, (chunk c, k-tile, w)]; (k, col) are the
            # logical XT k-tile / column; w must not straddle a 512 chunk
            c_, wo = divmod(col, 512)
            assert wo + w <= 512
            return xtall[:, ds(4096 * c_ + 512 * k + wo, w)]

        def xv_ap(j, col, w):
            return xvall[:, ds(CW * j + col, w)]

        # ---------------- S phase: running prefix sums S_t = sum_{j<=2t+1} K_j^T V_j
        # One Gram matmul per (pair, j): X_pair^T @ X_pair [128,128]; the two
        # diagonal 64x64 blocks are the per-head S states, off-diagonal blocks
        # are never read.  Snapshots after each outer block t (t=0..6).
        ssb = [[None] * (T - 1) for _ in range(2)]

        def emit_s_phase(pss):
            # j-outer so both pairs' Grams chase the incoming xv stream
            sps = [pss.tile([P, P], F32, name=f"sps{p}", tag=f"sps{p}")
                   for p in range(2)]
            for j in range(KB):
                for p in range(2):
                    v = xv_ap(j, P * p, P)
                    # skip_group_check: snapshots legitimately read the
                    # partial sum mid-accumulation-group (legal on HW)
                    nc.tensor.matmul(
                        sps[p], v, v, start=(j == 0), stop=(j == KB - 1),
                        skip_group_check=True,
                    )
                if j % 2 == 1 and j < KB - 1:
                    t_idx = j // 2
                    for p in range(2):
                        snap = cpool.tile(
                            [P, HD], DT, name=f"ssb{p}_{t_idx}", tag=f"ssb{p}_{t_idx}"
                        )
                        for e in range(2):
                            nc.vector.tensor_copy(
                                snap[ds(HD * e, HD), :],
                                sps[p][ds(HD * e, HD), ds(HD * e, HD)],
                            )
                        ssb[p][t_idx] = snap

        with tc.tile_pool(name="pss", bufs=1, space="PSUM") as pss:
            emit_s_phase(pss)
            qt_sb = [
                cpool.tile([P, N], DT, name=f"qt{m}", tag=f"qt{m}") for m in range(2)
            ]

        # ---------------- fused main loop over 512-column chunks c:
        #   Q-proj chunk c (both m halves), then attention blocks t=2c, 2c+1.
        # ST scores for both t's are emitted before the PV stage so the PE
        # has independent matmuls while the DVE does masked PSUM->SBUF copies.
        # o=0 block: full [128, 256] (left half tril-masked, right half dense).
        # o=1 block: only the right [128, 128] survives the mask (tril there).
        with tc.tile_pool(name="psst", bufs=3, space="PSUM") as psst:

            def emit_sts(p, t):
                out = []
                for o in range(2):
                    j = 2 * t + o
                    w_ = NQ if o == 0 else P
                    for e in range(2):
                        stp = psst.tile(
                            [P, NQ], F32, name=f"stp{p}_{t}_{o}_{e}", tag="stp"
                        )
                        c_, wo = divmod(P * j, 512)
                        kt = xtall[ds(HD * e, HD),
                                   ds(4096 * c_ + 512 * p + wo, P)]
                        qv = qt_sb[p][ds(HD * e, HD), ds(NQ * t + (NQ - w_), w_)]
                        nc.tensor.matmul(stp[:, :w_], kt, qv, start=True, stop=True,
                                         skip_group_check=True)
                        stsb = wpool.tile(
                            [P, NQ], DT, name=f"st{p}_{t}_{o}_{e}", tag="st",
                            bufs=16,
                        )
                        # causal mask fused into the PSUM->SBUF copy; the
                        # o=1 right half sees the same tril pattern as mk[:, :128]
                        mslice = mk_sb[:, :NQ] if o == 0 else mk_sb[:, :P]
                        nc.vector.tensor_mul(stsb[:, :w_], stp[:, :w_], mslice)
                        out.append((o, e, w_, stsb))
                return out

            def emit_pv(t, p, sts_tp):
                at = psat.tile([P, NQ], F32, name=f"at{p}_{t}", tag="at")

                # global term: attnT_t += S_{<t}^T @ Q_t^T (S symmetric)
                for e in range(2):
                    if t > 0:
                        nc.tensor.matmul(
                            at[ds(HD * e, HD), :],
                            ssb[p][t - 1][ds(HD * e, HD), :],
                            qt_sb[p][ds(HD * e, HD), ds(NQ * t, NQ)],
                            start=True, stop=False,
                            tile_position=(HD * e, HD * e),
                            # sim's coarse group check mishandles
                            # base_partition 64 slices; per-partition
                            # has_written semantics are correct
                            skip_group_check=True,
                        )

                # diagonal term: attnT_t += V_j^T @ ST_j
                for o, e, w_, stsb in sts_tp:
                    j = 2 * t + o
                    nc.tensor.matmul(
                        at[ds(HD * e, HD), ds(NQ - w_, w_)],
                        xv_ap(j, P * p + HD * e, HD),
                        stsb[:, :w_],
                        start=(t == 0 and o == 0), stop=(o == 1),
                        tile_position=(0, HD * e),
                        skip_group_check=True,
                    )

                ot = wpool.tile([P, NQ], F32, name=f"ot{p}_{t}", tag="ot")
                nc.scalar.copy(ot, at)
                nc.sync.dma_start(
                    out=out_d[ds(P * p, P), ds(NQ * t, NQ)], in_=ot
                )

            # two-stage pipeline across chunks: while the DVE masks chunk
            # c's scores, the PE runs chunk c-1's global/PV matmuls
            pending = []
            for c in range(4):
                # Q projection chunk c: qt[m][:, 512c:512c+512] = sum_k ...
                for m in range(2):
                    qp = psq.tile([P, 512], F32, name=f"qp{m}_{c}", tag=f"qp{m}")
                    for k in range(8):
                        nc.tensor.matmul(
                            qp,
                            wqall[:, ds(CW * k + P * m, P)],
                            xt_ap(k, 512 * c, 512),
                            start=(k == 0), stop=(k == 7),
                        )
                    nc.scalar.copy(qt_sb[m][:, ds(512 * c, 512)], qp)

                sts = []
                for t in (2 * c, 2 * c + 1):
                    for p in range(2):
                        sts.append((t, p, emit_sts(p, t)))

                for t, p, sts_tp in pending:
                    emit_pv(t, p, sts_tp)
                pending = sts

            for t, p, sts_tp in pending:
                emit_pv(t, p, sts_tp)


def build_nc(loop_n=1):
    nc = bacc.Bacc("TRN2", target_bir_lowering=False, debug=False)
    # all inputs ship pre-arranged in their SBUF layouts (see make_in_maps)
    xt_d = nc.dram_tensor("xt", [P, 8 * N], DT, kind="ExternalInput").ap()
    wq_d = nc.dram_tensor("wq", [P, 8 * CW], DT, kind="ExternalInput").ap()
    xv_d = nc.dram_tensor("xv", [P, KB * CW], DT, kind="ExternalInput").ap()
    mk_d = nc.dram_tensor("mk", [P, 2 * NQ], DT, kind="ExternalInput").ap()
    out_d = nc.dram_tensor("outT", [CW, N], F32, kind="ExternalOutput").ap()
    with tile.TileContext(nc) as tc:
        if loop_n > 1:
            # timing-only build: repeat the whole kernel on-device so the
            # per-iteration time can be separated from host/RPC overhead
            hints = (mybir.EngineType.PE, mybir.EngineType.DVE,
                     mybir.EngineType.Activation, mybir.EngineType.SP)
            with tc.For_i(0, loop_n, 1, hint_engines=hints):
                _emit(nc, tc, xt_d, wq_d, xv_d, mk_d, out_d)
        else:
            _emit(nc, tc, xt_d, wq_d, xv_d, mk_d, out_d)
    nc.compile()
    return nc


_CACHE = {}


def get_nc():
    if "nc" not in _CACHE:
        _CACHE["nc"] = build_nc()
    return _CACHE["nc"]


def make_in_maps(hidden_states, queries_weight):
    X = np.asarray(hidden_states, dtype=np.float32)
    W = np.asarray(queries_weight, dtype=np.float32)
    r = np.arange(P)[:, None]
    c = np.arange(NQ)[None, :]
    m0 = (c >= r).astype(np.float32)
    m1 = (c >= r + P).astype(np.float32)
    mk = np.concatenate([m0, m1], axis=1).astype(NPDT)
    in_maps = []
    for core in range(NCORES):
        b, g = divmod(core, 4)
        cols = slice(CW * g, CW * g + CW)
        # Permute the contraction rows so every core sees its own heads'
        # K^T rows at xt rows [0, 256) (keeps the program core-agnostic).
        perm = np.r_[
            np.arange(CW * g, CW * g + CW),
            np.arange(0, CW * g),
            np.arange(CW * g + CW, D),
        ]
        # pre-arrange into SBUF layouts so every DMA is fully contiguous:
        #   xt: [p, (chunk c, k-tile, w)], wq: [p, (k, w)], xv: [p, (j, w)]
        xt = (X[b].T[perm].reshape(8, P, 4, 512).transpose(1, 2, 0, 3)
              .reshape(P, 8 * N))
        wq = W[perm][:, cols].reshape(8, P, CW).transpose(1, 0, 2).reshape(P, 8 * CW)
        xv = X[b][:, cols].reshape(KB, P, CW).transpose(1, 0, 2).reshape(P, KB * CW)
        in_maps.append({
            "xt": np.ascontiguousarray(xt).astype(NPDT),
            "wq": np.ascontiguousarray(wq).astype(NPDT),
            "xv": np.ascontiguousarray(xv).astype(NPDT),
            "mk": mk,
        })
    return in_maps


def assemble(results):
    out = np.empty((B, N, D), dtype=np.float32)
    for core in range(NCORES):
        b, g = divmod(core, 4)
        out[b, :, CW * g:CW * g + CW] = results[core]["outT"].T
    return out


def kernel(hidden_states, queries_weight):
    nc = get_nc()
    in_maps = make_in_maps(hidden_states, queries_weight)
    res = bass_utils.run_bass_kernel_spmd(nc, in_maps, core_ids=list(range(NCORES)))
    return assemble(res.results)



# revision 52
# speedup vs baseline: 1.0329x; 1.0329x over previous
"""DenseAttention (causal quadratic variant, no softmax) — TRN2 Bass kernel.

Problem: out[b] = (tril(Q @ K^T) @ V) per head, where
  Q = X @ Wq (split into 16 heads of 64), K = V = X head slices.
Shapes: X [2, 2048, 1024] fp32, Wq [1024, 1024] fp32 -> out [2, 2048, 1024] fp32.

Sharding (8 cores): core c -> batch b = c//4, head group g = c%4 (4 heads,
output columns [256g, 256g+256)).  The queries projection is column-sharded
by head group; no cross-device communication.

Algorithm per core (linear-attention prefix-sum form, per head h, 128-row
blocks t):
  attn_t = Q_t @ S_{<t} + (tril(Q_t @ K_t^T) @ V_t)        [global + diagonal]
  S_t = S_{<t} + K_t^T @ V_t                               [64x64 state/head]
All second-stage matmuls run "flipped" (scores / Q^T stationary) so the
moving stream is only 64-128 columns; output comes out directly in [n, d]
layout and ships as bf16 (host upcasts).

HW constraint discovered on TRN2: matmuls with different tile_position rows
may not write full-partition outputs into the same PSUM bank.  Hence the
score tile is split per-e into two banks ([P, 2, 512] 3D tile) and the
global term uses full-128 contraction against zero-padded S states so every
at-bank writer is tile_position (0, 0).

All matmuls run in bf16 with fp32 PSUM accumulation.
"""

import numpy as np
import ml_dtypes

import concourse.bacc as bacc
import concourse.mybir as mybir
import concourse.tile as tile
from concourse import bass_utils
from concourse.bass import ds

B, N, D = 2, 2048, 1024
H, HD = 16, 64
NCORES = 8
P = 128           # partition dim == block size
T = N // P        # 16 blocks
CW = 256          # per-core output column width (4 heads)

DT = mybir.dt.bfloat16
NPDT = ml_dtypes.bfloat16
F32 = mybir.dt.float32


def _emit(nc, tc, xt_d, wq_d, kt_d, xv_d, mk_d, out_d):
    with (
        tc.tile_pool(name="const", bufs=1) as cpool,
        tc.tile_pool(name="work", bufs=8) as wpool,
        tc.tile_pool(name="psq", bufs=1, space="PSUM") as psq,
    ):
        # ---------------- input DMAs (all fully contiguous row-slices).
        # Two HWDGE queues, ordered by first PE consumption:
        #   ACT: wq, xt c0 (2 halves), mk, xt c2       (then ACT does copies)
        #   SP:  xv (2), kt (2), xt c1, xv (2), xt c3, out DMAs
        wqall = cpool.tile([P, 16 * P], DT, name="wqall", tag="wqall")
        nc.scalar.dma_start(out=wqall, in_=wq_d)

        xvall = cpool.tile([P, T * CW], DT, name="xvall", tag="xvall")
        xtall = cpool.tile([P, 6 * N], DT, name="xtall", tag="xtall")
        ktall = cpool.tile([P, 2 * N], DT, name="ktall", tag="ktall")
        mk_sb = cpool.tile([P, 512], DT, name="mk_sb", tag="mk_sb")

        # xt carries only the 6 "foreign" k-tiles per n-chunk (3072 cols per
        # chunk); the core's own two k-tiles of X^T are read from ktall.
        nc.sync.dma_start(out=xvall[:, ds(0, 1024)], in_=xv_d[:, ds(0, 1024)])
        nc.sync.dma_start(out=xvall[:, ds(1024, 1024)], in_=xv_d[:, ds(1024, 1024)])
        nc.sync.dma_start(out=ktall[:, ds(0, 2048)], in_=kt_d[:, ds(0, 2048)])
        nc.sync.dma_start(out=ktall[:, ds(2048, 2048)], in_=kt_d[:, ds(2048, 2048)])
        nc.sync.dma_start(out=xtall[:, ds(3072, 3072)], in_=xt_d[:, ds(3072, 3072)])
        nc.sync.dma_start(out=xvall[:, ds(2048, 1024)], in_=xv_d[:, ds(2048, 1024)])
        nc.sync.dma_start(out=xvall[:, ds(3072, 1024)], in_=xv_d[:, ds(3072, 1024)])
        nc.sync.dma_start(out=xtall[:, ds(9216, 3072)], in_=xt_d[:, ds(9216, 3072)])

        nc.scalar.dma_start(out=xtall[:, ds(0, 1536)], in_=xt_d[:, ds(0, 1536)])
        nc.scalar.dma_start(out=xtall[:, ds(1536, 1536)], in_=xt_d[:, ds(1536, 1536)])
        nc.scalar.dma_start(out=mk_sb, in_=mk_d)
        nc.scalar.dma_start(out=xtall[:, ds(6144, 3072)], in_=xt_d[:, ds(6144, 3072)])

        def xv_ap(j, col, w):
            return xvall[:, ds(CW * j + col, w)]

        # ---------------- S states.  Per block j one fresh PSUM Gram tile
        # gp [128,128] (head (p,e) block at rows 64e, cols 64p — writers are
        # partition-disjoint across e so the bank is legal), then a DVE
        # prefix accumulation into ZERO-PADDED bf16 states: snall column
        # group 64*(2e+p) of slot j holds S(p,e) on rows 64e and zeros
        # elsewhere, so the global matmul can contract over the full 128
        # partitions with tile_position (0,0).  Two strided DVE adds per j.
        snall = cpool.tile([P, (T - 1) * CW], DT, name="snall", tag="snall")
        nc.vector.memset(snall, 0.0)

        def sn_ap(j, e):
            # cols [256j + 128e, +128) on rows [64e, +64): (p0|p1) for e
            return snall[ds(HD * e, HD), ds(CW * j + P * e, P)]

        def emit_gram(psg, j):
            gp = psg.tile([P, P], F32, name=f"g_{j}", tag="g")
            for p in range(2):
                for e in range(2):
                    v = xv_ap(j, P * p + HD * e, HD)
                    nc.tensor.matmul(
                        gp[ds(HD * e, HD), ds(HD * p, HD)], v, v,
                        start=True, stop=True,
                        tile_position=(0, HD * e), skip_group_check=True,
                    )
            for e in range(2):
                if j == 0:
                    nc.vector.tensor_copy(sn_ap(0, e), gp[ds(HD * e, HD), :])
                else:
                    nc.vector.tensor_add(
                        sn_ap(j, e), sn_ap(j - 1, e), gp[ds(HD * e, HD), :])

        qt_sb = [
            cpool.tile([P, N], DT, name=f"qt{p}", tag=f"qt{p}") for p in range(2)
        ]

        def emit_qproj(c, p):
            # qt[p][:, 512c:+512] = sum_k wq[k,p]^T @ xt[c,k].  Contraction
            # k-tiles are host-permuted so k=0,1 are the core's own head
            # dims — their X^T rows stream from ktall instead of xtall.
            qp = psq.tile([P, 512], F32, name=f"qp{p}_{c}", tag="qp")
            for k in range(8):
                if k < 2:
                    rhs = ktall[:, ds(2048 * k + 512 * c, 512)]
                else:
                    rhs = xtall[:, ds(3072 * c + 512 * (k - 2), 512)]
                nc.tensor.matmul(
                    qp,
                    wqall[:, ds(P * (2 * k + p), P)],
                    rhs,
                    start=(k == 0), stop=(k == 7),
                )
            nc.scalar.copy(qt_sb[p][:, ds(512 * c, 512)], qp)

        def emit_st(t):
            # scores^T per head into a 3D [P, 2, 512] tile: e selects the
            # PSUM bank (one tile_position row per bank), pair p at col
            # 128p.  One batched strided mask-multiply -> stsb cols
            # 128*(2e+p).
            stp = psst.tile([P, 2, 512], F32, name=f"st{t}", tag="stp")
            for p in range(2):
                for e in range(2):
                    nc.tensor.matmul(
                        stp[:, e, ds(P * p, P)],
                        ktall[ds(HD * e, HD), ds(N * p + P * t, P)],
                        qt_sb[p][ds(HD * e, HD), ds(P * t, P)],
                        start=(p == 0), stop=True,
                        tile_position=(HD * e, 0), skip_group_check=True,
                    )
            stsb = wpool.tile([P, 512], DT, name=f"sb{t}", tag="st", bufs=8)
            nc.vector.tensor_mul(stsb, stp[:, :, ds(0, 256)], mk_sb)
            return stsb

        def emit_global(t, at):
            # at += Q_t @ S_{<t}: full-128 contraction (zero-padded rhs),
            # tile_position (0,0) like every other at-bank writer.  One
            # PSUM start per at pair-bank: its first writer.
            atp, base = at
            first = t == 1 or t % 2 == 0
            for p in range(2):
                for e in range(2):
                    nc.tensor.matmul(
                        atp[:, ds(base + HD * (2 * p + e), HD)],
                        qt_sb[p][:, ds(P * t, P)],
                        snall[:, ds(CW * (t - 1) + HD * (2 * e + p), HD)],
                        start=(first and p == 0 and e == 0), stop=False,
                        skip_group_check=True,
                    )

        def emit_pv(t, at, stsb):
            atp, base = at
            for p in range(2):
                for e in range(2):
                    nc.tensor.matmul(
                        atp[:, ds(base + HD * (2 * p + e), HD)],
                        stsb[:, ds(P * (2 * e + p), P)],
                        xv_ap(t, P * p + HD * e, HD),
                        start=False, stop=True,
                        skip_group_check=True,
                    )
            ot = wpool.tile([P, CW], DT, name=f"ot{t}", tag="ot", bufs=16)
            nc.scalar.copy(ot, atp[:, ds(base, CW)])
            nc.sync.dma_start(out=out_d[ds(P * t, P), :], in_=ot)

        # ---------------- emission.  Prologue: all grams (scoped PSUM
        # pool; the DVE snap chain runs behind, under the DMA-bound
        # window), the two qproj(0) halves interleaved.  Main loop: per
        # block — ST(t)+global(t), PV(t-3)+out, then any scheduled qproj
        # half (the ACT qt copy lands blocks before ST(4c) reads it).
        # PV lags ST by 3 blocks so the batched DVE mask multiply is off
        # the PE's critical path.
        with tc.tile_pool(name="psg", bufs=6, space="PSUM") as psg:
            for j in range(3):
                emit_gram(psg, j)
            emit_qproj(0, 0)
            for j in range(3, T - 1):
                emit_gram(psg, j)
            emit_qproj(0, 1)
        qsched = {2: (1, 0), 3: (1, 1), 5: (2, 0), 6: (2, 1), 9: (3, 0), 10: (3, 1)}
        with (
            tc.tile_pool(name="psst", bufs=2, space="PSUM") as psst,
            tc.tile_pool(name="psat", bufs=3, space="PSUM") as psat,
        ):
            pending = []  # (t, at, stsb)
            atp = None
            for t in range(T):
                if t % 2 == 0:
                    # two blocks share one PSUM bank -> 6 blocks of ring slack
                    atp = psat.tile([P, 512], F32, name=f"at{t}", tag="at")
                at = (atp, CW * (t % 2))
                stsb = emit_st(t)
                if t > 0:
                    emit_global(t, at)
                pending.append((t, at, stsb))
                if len(pending) > 3:
                    emit_pv(*pending.pop(0))
                if t in qsched:
                    emit_qproj(*qsched[t])
            while pending:
                emit_pv(*pending.pop(0))


def build_nc(loop_n=1):
    nc = bacc.Bacc("TRN2", target_bir_lowering=False, debug=False)
    # all inputs ship pre-arranged in their SBUF layouts (see make_in_maps)
    xt_d = nc.dram_tensor("xt", [P, 6 * N], DT, kind="ExternalInput").ap()
    wq_d = nc.dram_tensor("wq", [P, 16 * P], DT, kind="ExternalInput").ap()
    kt_d = nc.dram_tensor("kt", [P, 2 * N], DT, kind="ExternalInput").ap()
    xv_d = nc.dram_tensor("xv", [P, T * CW], DT, kind="ExternalInput").ap()
    mk_d = nc.dram_tensor("mk", [P, 512], DT, kind="ExternalInput").ap()
    out_d = nc.dram_tensor("outQ", [N, CW], DT, kind="ExternalOutput").ap()
    with tile.TileContext(nc) as tc:
        if loop_n > 1:
            # timing-only build: repeat the whole kernel on-device so the
            # per-iteration time can be separated from host/RPC overhead
            hints = (mybir.EngineType.PE, mybir.EngineType.DVE,
                     mybir.EngineType.Activation, mybir.EngineType.SP)
            with tc.For_i(0, loop_n, 1, hint_engines=hints):
                _emit(nc, tc, xt_d, wq_d, kt_d, xv_d, mk_d, out_d)
        else:
            _emit(nc, tc, xt_d, wq_d, kt_d, xv_d, mk_d, out_d)
    nc.compile()
    return nc


_CACHE = {}


def get_nc():
    if "nc" not in _CACHE:
        _CACHE["nc"] = build_nc()
    return _CACHE["nc"]


def make_in_maps(hidden_states, queries_weight):
    X = np.asarray(hidden_states, dtype=np.float32)
    W = np.asarray(queries_weight, dtype=np.float32)
    r = np.arange(P)[:, None]
    c = np.arange(P)[None, :]
    mk = np.tile((c >= r).astype(NPDT), (1, 4))
    in_maps = []
    for core in range(NCORES):
        b, g = divmod(core, 4)
        cols = slice(CW * g, CW * g + CW)
        Xb = X[b]
        # pre-arrange into SBUF layouts so every DMA is fully contiguous.
        # Contraction rows are permuted own-head-dims-first so the Q-proj's
        # first two k-tiles alias ktall (the program is core-agnostic):
        #   xt: [p, (c, k6, 512)] = foreign X^T k-tiles, n-chunk cols
        #   wq: [p, (k, p2, 128)] = permuted Wq k-tile rows, head-pair cols
        #   kt: [p, (pair, n)]    = own head dims ^T (ST lhsT + Q-proj rhs)
        #   xv: [p, (j, 256)]     = own head cols, 128-row blocks (V / Gram)
        perm = np.r_[np.arange(CW * g, CW * g + CW),
                     np.arange(0, CW * g), np.arange(CW * g + CW, D)]
        xt = (Xb.T[perm[CW:]].reshape(6, P, 4, 512).transpose(1, 2, 0, 3)
              .reshape(P, 6 * N))
        wq = (W[perm][:, cols].reshape(8, P, 2, P).transpose(1, 0, 2, 3)
              .reshape(P, 16 * P))
        kt = (Xb[:, cols].T.reshape(2, P, N).transpose(1, 0, 2).reshape(P, 2 * N))
        xv = Xb[:, cols].reshape(T, P, CW).transpose(1, 0, 2).reshape(P, T * CW)
        in_maps.append({
            "xt": np.ascontiguousarray(xt).astype(NPDT),
            "wq": np.ascontiguousarray(wq).astype(NPDT),
            "kt": np.ascontiguousarray(kt).astype(NPDT),
            "xv": np.ascontiguousarray(xv).astype(NPDT),
            "mk": mk,
        })
    return in_maps


def assemble(results):
    out = np.empty((B, N, D), dtype=np.float32)
    for core in range(NCORES):
        b, g = divmod(core, 4)
        out[b, :, CW * g:CW * g + CW] = results[core]["outQ"].astype(np.float32)
    return out


def kernel(hidden_states, queries_weight):
    nc = get_nc()
    in_maps = make_in_maps(hidden_states, queries_weight)
    res = bass_utils.run_bass_kernel_spmd(nc, in_maps, core_ids=list(range(NCORES)))
    return assemble(res.results)
